# revision 12
# baseline (speedup 1.0000x reference)
"""DFA block kernel for Trainium2 (raw Bass), SPMD over 8 NeuronCores.

The reference op (window-partition -> per-channel 1x1 depthwise affine ->
window-reverse -> residual add) reduces, for H=W=320 / WN=5 (64x64 windows,
no padding), to a pure elementwise affine with block-structured coefficients:

    out[b, c, h, w] = x[b, c, h, w] * (w[k] + 1) + b[k]
    k = c*25 + (h // 64)*5 + (w // 64)

Sharding: data-parallel over batch B=8, one batch element per core; the 1600
coefficient floats are replicated.

Per-core layout: x[b] viewed as groups g = (c, i=h//64) -> 320 groups, each a
contiguous run of 64 rows x 320 cols (81,920 B).  SBUF tiles take up to 128
groups on partitions and a 16-row slab of each group's rows in the free dim,
so every DMA moves large fully-contiguous per-partition runs.  The affine is
one fp32 tensor_scalar per column block j (scale/bias are [P,1] per-partition
scalars), in place.

Raw-Bass 3-engine pipeline (Tile's auto-sems overflow walrus's tiny
per-instruction sync-wait budget, so semaphores are explicit):
  SP   (nc.sync)   : loads   x -> slot          (HWDGE ring 1)
  ACT  (nc.scalar) : stores  slot -> y          (HWDGE ring 2)
  DVE  (nc.vector) : 5x tensor_scalar per slot, in place
"""

import numpy as np

import concourse.bass as bass
import concourse.mybir as mybir
from concourse.bass_utils import run_bass_kernel_spmd

B, C, H, W = 8, 64, 320, 320
WN = 5           # windows per axis
HS = H // WN     # 64 rows per window
G = C * WN       # 320 (c, i) groups per core
N_CORES = 8
F32 = mybir.dt.float32

HS_CHUNK = 16            # rows of W floats per tile slab
N_K = HS // HS_CHUNK     # 4 slabs per group
CHUNKS = [(0, 128), (128, 128), (256, 64)]
TILES = [(ci, g0, p, k) for ci, (g0, p) in enumerate(CHUNKS) for k in range(N_K)]
NT = len(TILES)          # 12
NSLOT = 6
# packed coefficient layout: for chunk ci, cols [ci*10, ci*10+5) = scale,
# [ci*10+5, ci*10+10) = bias; rows = partition within chunk
COEF_COLS = len(CHUNKS) * 2 * WN


def _build_bass(repeat=1):
    """repeat>1 builds a benchmark variant that runs the whole pipeline
    `repeat` times inside one NEFF (same tiles, cumulative sem values), so
    device time scales with `repeat` while host dispatch stays constant."""
    nc = bass.Bass()
    x = nc.dram_tensor("x", (G, N_K, HS_CHUNK, WN, HS), F32, kind="ExternalInput")
    coef = nc.dram_tensor("coef", (128, COEF_COLS), F32, kind="ExternalInput")
    y = nc.dram_tensor("y", (G, N_K, HS_CHUNK, WN, HS), F32, kind="ExternalOutput")

    from contextlib import ExitStack

    with ExitStack() as stack:
        ct = stack.enter_context(nc.sbuf_tensor([128, COEF_COLS], F32))
        data = stack.enter_context(
            nc.sbuf_tensor([128, NSLOT, HS_CHUNK, WN, HS], F32)
        )
        # One semaphore per DMA: HWDGE dma_starts spread over multiple
        # logical queues and can complete OUT OF ORDER across instructions,
        # so a cumulative per-ring counter is unsound (observed on HW as
        # stale-tile reads).
        in_sems = [
            stack.enter_context(nc.semaphore(f"in_sem{n}")) for n in range(NT)
        ]
        out_sems = [
            stack.enter_context(nc.semaphore(f"out_sem{n}")) for n in range(NT)
        ]
        dve_sem = stack.enter_context(nc.semaphore("dve_sem"))
        coef_sem = stack.enter_context(nc.semaphore("coef_sem"))
        block = stack.enter_context(nc.Block())

        @block.sync
        def _(sync):
            sync.dma_start(out=ct[:, :], in_=coef[:, :]).then_inc(coef_sem, 16)
            for r in range(repeat):
                for n, (ci, g0, p, k) in enumerate(TILES):
                    m = r * NT + n
                    s = m % NSLOT
                    if m >= NSLOT:
                        # slot reuse: wait until store m-NSLOT read it out
                        pn, pr = (m - NSLOT) % NT, (m - NSLOT) // NT
                        sync.wait_ge(out_sems[pn], 16 * (pr + 1))
                    sync.dma_start(
                        out=data[:p, s], in_=x[g0 : g0 + p, k]
                    ).then_inc(in_sems[n], 16)

        @block.vector
        def _(vector):
            vector.wait_ge(coef_sem, 16)
            for r in range(repeat):
                for n, (ci, g0, p, k) in enumerate(TILES):
                    m = r * NT + n
                    s = m % NSLOT
                    vector.wait_ge(in_sems[n], 16 * (r + 1))
                    if m >= NSLOT:
                        # in-place write over a slot the store engine read
                        pn, pr = (m - NSLOT) % NT, (m - NSLOT) // NT
                        vector.wait_ge(out_sems[pn], 16 * (pr + 1))
                    for j in range(WN):
                        vector.tensor_scalar(
                            data[:p, s, :, j, :],
                            data[:p, s, :, j, :],
                            ct[:p, ci * 10 + j : ci * 10 + j + 1],
                            ct[:p, ci * 10 + WN + j : ci * 10 + WN + j + 1],
                            mybir.AluOpType.mult,
                            mybir.AluOpType.add,
                        ).then_inc(dve_sem, 1)

        @block.scalar
        def _(scalar):
            for r in range(repeat):
                for n, (ci, g0, p, k) in enumerate(TILES):
                    m = r * NT + n
                    s = m % NSLOT
                    scalar.wait_ge(dve_sem, WN * (m + 1))
                    scalar.dma_start(
                        out=y[g0 : g0 + p, k], in_=data[:p, s]
                    ).then_inc(out_sems[n], 16)

    return nc


_NC_CACHE = {}


def _get_nc():
    if "nc" not in _NC_CACHE:
        _NC_CACHE["nc"] = _build_bass()
    return _NC_CACHE["nc"]


def _make_in_maps(x, w, b):
    # scale/bias indexed by flat k = g*WN + j with g = c*WN + i
    s_arr = (w + 1.0).reshape(G, WN).astype(np.float32)
    b_arr = b.reshape(G, WN).astype(np.float32)
    coef = np.zeros((128, COEF_COLS), dtype=np.float32)
    for ci, (g0, p) in enumerate(CHUNKS):
        coef[:p, ci * 10 : ci * 10 + WN] = s_arr[g0 : g0 + p]
        coef[:p, ci * 10 + WN : ci * 10 + 2 * WN] = b_arr[g0 : g0 + p]
    in_maps = []
    for core in range(N_CORES):
        xs = np.ascontiguousarray(
            x[core]
            .reshape(C, WN, N_K, HS_CHUNK, WN, HS)
            .reshape(G, N_K, HS_CHUNK, WN, HS)
        )
        in_maps.append({"x": xs, "coef": coef})
    return in_maps


def _assemble(results):
    outs = []
    for core in range(N_CORES):
        yc = results[core]["y"].reshape(C, WN, HS, W).reshape(C, H, W)
        outs.append(yc)
    return np.stack(outs, axis=0)


def run(x, w, b, **spmd_kwargs):
    """Run the SPMD kernel; returns (output, BassKernelResults)."""
    nc = _get_nc()
    res = run_bass_kernel_spmd(
        nc, _make_in_maps(x, w, b), core_ids=list(range(N_CORES)), **spmd_kwargs
    )
    return _assemble(res.results), res


def kernel(x, w, b):
    x = np.asarray(x, dtype=np.float32)
    w = np.asarray(w, dtype=np.float32)
    b = np.asarray(b, dtype=np.float32)
    out, _ = run(x, w, b)
    return out


# revision 16
# speedup vs baseline: 1.0321x; 1.0321x over previous
"""DFA block kernel for Trainium2 (raw Bass), SPMD over 8 NeuronCores.

The reference op (window-partition -> per-channel 1x1 depthwise affine ->
window-reverse -> residual add) reduces, for H=W=320 / WN=5 (64x64 windows,
no padding), to a pure elementwise affine with block-structured coefficients:

    out[b, c, h, w] = x[b, c, h, w] * (w[k] + 1) + b[k]
    k = c*25 + (h // 64)*5 + (w // 64)

Sharding: data-parallel over batch B=8, one batch element per core; the 1600
coefficient floats are replicated.

Per-core layout: x[b] viewed as groups g = (c, i=h//64) -> 320 groups, each a
contiguous run of 64 rows x 320 cols (81,920 B).  SBUF tiles take up to 128
groups on partitions and a 16-row slab of each group's rows in the free dim,
so every DMA moves large fully-contiguous per-partition runs.  The affine is
one fp32 tensor_scalar per column block j (scale/bias are [P,1] per-partition
scalars), in place.

Raw-Bass 3-engine pipeline (Tile's auto-sems overflow walrus's tiny
per-instruction sync-wait budget, so semaphores are explicit):
  SP   (nc.sync)   : loads   x -> slot          (HWDGE ring 1)
  ACT  (nc.scalar) : stores  slot -> y          (HWDGE ring 2)
  DVE  (nc.vector) : 5x tensor_scalar per slot, in place
"""

import numpy as np

import concourse.bass as bass
import concourse.mybir as mybir
from concourse.bass_utils import run_bass_kernel_spmd

B, C, H, W = 8, 64, 320, 320
WN = 5           # windows per axis
HS = H // WN     # 64 rows per window
G = C * WN       # 320 (c, i) groups per core
N_CORES = 8
F32 = mybir.dt.float32

HS_CHUNK = 16            # max rows of W floats per tile slab
N_K = HS // HS_CHUNK     # kept for the harness-side packing helpers
CHUNKS = [(0, 128), (128, 128), (256, 64)]
# Per-chunk row splits.  The first tile of chunk 0 and the last tiles of
# chunk 2 are small so the pipeline head (first load before compute can
# start) and tail (last store after compute ends) are short.
_ROW_SPLITS = [
    [4, 12, 16, 16, 16],   # chunk 0 (head ramp)
    [16, 16, 16, 16],      # chunk 1
    [16, 16, 16, 12, 4],   # chunk 2 (tail ramp; p=64 so already half-size)
]
# tile = (ci, g0, p, r0, rows)
TILES = []
for ci, (g0, p) in enumerate(CHUNKS):
    r0 = 0
    for rows in _ROW_SPLITS[ci]:
        TILES.append((ci, g0, p, r0, rows))
        r0 += rows
    assert r0 == HS
NT = len(TILES)          # 14
NSLOT = 6
# packed coefficient layout: for chunk ci, cols [ci*10, ci*10+5) = scale,
# [ci*10+5, ci*10+10) = bias; rows = partition within chunk
COEF_COLS = len(CHUNKS) * 2 * WN


def _build_bass(repeat=1):
    """repeat>1 builds a benchmark variant that runs the whole pipeline
    `repeat` times inside one NEFF (same tiles, cumulative sem values), so
    device time scales with `repeat` while host dispatch stays constant."""
    nc = bass.Bass()
    x = nc.dram_tensor("x", (G, HS, WN, HS), F32, kind="ExternalInput")
    coef = nc.dram_tensor("coef", (128, COEF_COLS), F32, kind="ExternalInput")
    y = nc.dram_tensor("y", (G, HS, WN, HS), F32, kind="ExternalOutput")

    from contextlib import ExitStack

    with ExitStack() as stack:
        ct = stack.enter_context(nc.sbuf_tensor([128, COEF_COLS], F32))
        data = stack.enter_context(
            nc.sbuf_tensor([128, NSLOT, HS_CHUNK, WN, HS], F32)
        )
        # One semaphore per DMA: HWDGE dma_starts spread over multiple
        # logical queues and can complete OUT OF ORDER across instructions,
        # so a cumulative per-ring counter is unsound (observed on HW as
        # stale-tile reads).
        in_sems = [
            stack.enter_context(nc.semaphore(f"in_sem{n}")) for n in range(NT)
        ]
        out_sems = [
            stack.enter_context(nc.semaphore(f"out_sem{n}")) for n in range(NT)
        ]
        dve_sem = stack.enter_context(nc.semaphore("dve_sem"))
        coef_sem = stack.enter_context(nc.semaphore("coef_sem"))
        block = stack.enter_context(nc.Block())

        @block.sync
        def _(sync):
            for r in range(repeat):
                for n, (ci, g0, p, r0, rows) in enumerate(TILES):
                    m = r * NT + n
                    s = m % NSLOT
                    if m >= NSLOT:
                        # slot reuse: wait until store m-NSLOT read it out
                        pn, pr = (m - NSLOT) % NT, (m - NSLOT) // NT
                        sync.wait_ge(out_sems[pn], 16 * (pr + 1))
                    sync.dma_start(
                        out=data[:p, s, :rows], in_=x[g0 : g0 + p, r0 : r0 + rows]
                    ).then_inc(in_sems[n], 16)

        @block.vector
        def _(vector):
            vector.wait_ge(coef_sem, 16)
            for r in range(repeat):
                for n, (ci, g0, p, r0, rows) in enumerate(TILES):
                    m = r * NT + n
                    s = m % NSLOT
                    vector.wait_ge(in_sems[n], 16 * (r + 1))
                    if m >= NSLOT:
                        # in-place write over a slot the store engine read
                        pn, pr = (m - NSLOT) % NT, (m - NSLOT) // NT
                        vector.wait_ge(out_sems[pn], 16 * (pr + 1))
                    for j in range(WN):
                        vector.tensor_scalar(
                            data[:p, s, :rows, j, :],
                            data[:p, s, :rows, j, :],
                            ct[:p, ci * 10 + j : ci * 10 + j + 1],
                            ct[:p, ci * 10 + WN + j : ci * 10 + WN + j + 1],
                            mybir.AluOpType.mult,
                            mybir.AluOpType.add,
                        ).then_inc(dve_sem, 1)

        @block.scalar
        def _(scalar):
            # coef load rides the store engine's ring, which is idle at the
            # head, so it does not delay the first data load
            scalar.dma_start(out=ct[:, :], in_=coef[:, :]).then_inc(coef_sem, 16)
            for r in range(repeat):
                for n, (ci, g0, p, r0, rows) in enumerate(TILES):
                    m = r * NT + n
                    s = m % NSLOT
                    scalar.wait_ge(dve_sem, WN * (m + 1))
                    scalar.dma_start(
                        out=y[g0 : g0 + p, r0 : r0 + rows], in_=data[:p, s, :rows]
                    ).then_inc(out_sems[n], 16)

    return nc


_NC_CACHE = {}


def _get_nc():
    if "nc" not in _NC_CACHE:
        _NC_CACHE["nc"] = _build_bass()
    return _NC_CACHE["nc"]


def _make_in_maps(x, w, b):
    # scale/bias indexed by flat k = g*WN + j with g = c*WN + i
    s_arr = (w + 1.0).reshape(G, WN).astype(np.float32)
    b_arr = b.reshape(G, WN).astype(np.float32)
    coef = np.zeros((128, COEF_COLS), dtype=np.float32)
    for ci, (g0, p) in enumerate(CHUNKS):
        coef[:p, ci * 10 : ci * 10 + WN] = s_arr[g0 : g0 + p]
        coef[:p, ci * 10 + WN : ci * 10 + 2 * WN] = b_arr[g0 : g0 + p]
    in_maps = []
    for core in range(N_CORES):
        # (C, H, W) -> (c, i, hs, j, ws) -> (g, hs, j, ws); pure reshapes
        xs = np.ascontiguousarray(
            x[core].reshape(C, WN, HS, WN, HS).reshape(G, HS, WN, HS)
        )
        in_maps.append({"x": xs, "coef": coef})
    return in_maps


def _assemble(results):
    outs = []
    for core in range(N_CORES):
        yc = results[core]["y"].reshape(C, WN, HS, W).reshape(C, H, W)
        outs.append(yc)
    return np.stack(outs, axis=0)


def run(x, w, b, **spmd_kwargs):
    """Run the SPMD kernel; returns (output, BassKernelResults)."""
    nc = _get_nc()
    res = run_bass_kernel_spmd(
        nc, _make_in_maps(x, w, b), core_ids=list(range(N_CORES)), **spmd_kwargs
    )
    return _assemble(res.results), res


def kernel(x, w, b):
    x = np.asarray(x, dtype=np.float32)
    w = np.asarray(w, dtype=np.float32)
    b = np.asarray(b, dtype=np.float32)
    out, _ = run(x, w, b)
    return out
